# revision 24
# baseline (speedup 1.0000x reference)
"""DeepFactorRNN Trainium2 kernel.

Computes, for x = X.reshape(-1, F):
  mus    = sum_j(relu(LSTM2g(LSTM1g(x))) @ aff_W.T + aff_b)_j
  sigmas = softplus(relu(LSTM2n(LSTM1n(x))) @ noise_W.T + noise_b) + 1e-6
where each LSTM is a single step from zero state (so the forget gate is
unused and c = sigmoid(i)*tanh(g), h = sigmoid(o)*tanh(c)).

Strategy (8 NeuronCores, data parallel over the 32768 flattened rows):
 - Rows live on the matmul free dim; features/gates on partitions, so the
   whole network is transpose-free. X is transposed/cast on host.
 - f-gates are dropped from all weight matrices (25% matmul savings).
 - The aff linear + sum collapses to one dot with w_mu = aff_W.sum(0).
 - bf16 matmul operands, fp32 PSUM accumulation, fp32 activation math.
 - ACT keeps only the 3 gate table passes per chunk (its per-element
   floor); tanh(c) runs on the DVE (|c|<=1) as least-squares polys fitted
   to the empirical c distribution:
     layer 0:  c*(PD0 + PD1*c^2)  2 tensor_tensor + 1 tensor_scalar
     layer 1:  c*(A0  + A1*c)     1 tensor_tensor + 1 fused tensor_scalar
   The layer-1 quad needs no max/relu stage: on [-1,0) A0+A1*c > 0 keeps
   sign(h) == sign(c), so the single final relu on h reproduces
   tanh(relu(c)) exactly while fitting the c>=0 half-range better than an
   odd cubic. scalar_tensor_tensor is never used (it runs at DVE 1x mode;
   tensor_scalar runs 4x, tensor_tensor 2x).
 - Work stays at single-chunk granularity ([128, RT] tiles): wider fused
   tiles serialize the engines through the 8-bank PSUM, and measurably
   denser engine overlap trips a chip-wide ~1.2x clock throttle (observed
   deterministically per compiled NEFF), so instruction savings are taken
   only where they do not tighten simultaneous engine activity further.
 - Emission is software-pipelined with a one-tile skew: tile t layer-0
   chunks (ACT-heavy) interleave with tile t-1 layer-1 chunks (PE-heavy)
   so all engine queues stay dense.
 - Tail row-sums for mu and sigma share one PSUM tile (sigma lands at
   partition 32 via tile_position) so one ACT staging copy serves both,
   landing in ACT idle windows; the last tile splits the copies so the mu
   DMA overlaps the remaining noise compute during the drain.
 - A dummy activation with no DMA deps hoists the ~1.3us ACT table load
   into the preamble; the noise branch is computed first at startup (its
   weights are 4x smaller) so the PE starts while w0g still loads; tails
   are emitted a full round late so their matmuls never head-of-line
   block the PE queue.
 - The constant aff bias and the softplus epilogue fold on the host.
"""

from functools import partial

import numpy as np
import ml_dtypes

BF16 = ml_dtypes.bfloat16

NCORES = 8
NTS, NPER, F = 128, 256, 128
GH, NH = 512, 256
ROWS = NTS * NPER            # 32768
RPC = ROWS // NCORES         # 4096 rows per core
RT = 1024                    # rows per tile
NT = RPC // RT               # 4 tiles per core
HALF = 512                   # matmul moving max: one PSUM bank of fp32

# tanh(x) ~= x*(PD0 + PD1*x^2), least-squares fit over the empirical
# c = sigmoid(i)*tanh(g) distribution of all four LSTM layers
PD0, PD1 = 0.9925682, -0.26160714
# tanh(x) ~= x*(A0 + A1*x) for x >= 0, fit per-branch over the relu'd
# layer-1 c distribution (used with the relu folded into the max op;
# on the c>=0 half-range this quad fit beats the odd cubic)
ABg = (1.0057485, -0.09441389)
ABn = (1.0060241, -0.09650258)

_CACHE = {}


def _build_program():
    import concourse.bacc as bacc
    import concourse.tile as tile
    from concourse import mybir

    dt = mybir.dt
    AFT = mybir.ActivationFunctionType
    ALU = mybir.AluOpType

    nc = bacc.Bacc("TRN2", target_bir_lowering=False, debug=False,
                   num_devices=NCORES)

    # ---- DRAM I/O ----
    d_xT = nc.dram_tensor("xT", [F, RPC], dt.bfloat16, kind="ExternalInput")
    d_w0g = nc.dram_tensor("w0g", [F, 3 * GH], dt.bfloat16, kind="ExternalInput")
    d_w1g = nc.dram_tensor("w1g", [GH, 3 * GH], dt.bfloat16, kind="ExternalInput")
    d_w0n = nc.dram_tensor("w0n", [F, 3 * NH], dt.bfloat16, kind="ExternalInput")
    d_w1n = nc.dram_tensor("w1n", [NH, 3 * NH], dt.bfloat16, kind="ExternalInput")
    d_wmu = nc.dram_tensor("wmu", [128, GH // 128], dt.bfloat16, kind="ExternalInput")
    d_wsig = nc.dram_tensor("wsig", [128, NH // 128], dt.bfloat16, kind="ExternalInput")
    d_bg0 = nc.dram_tensor("bg0", [128, 3 * GH // 128], dt.float32, kind="ExternalInput")
    d_bg1 = nc.dram_tensor("bg1", [128, 3 * GH // 128], dt.float32, kind="ExternalInput")
    d_bn0 = nc.dram_tensor("bn0", [128, 3 * NH // 128], dt.float32, kind="ExternalInput")
    d_bn1 = nc.dram_tensor("bn1", [128, 3 * NH // 128], dt.float32, kind="ExternalInput")
    d_mus = nc.dram_tensor("mus_o", [1, RPC], dt.float32, kind="ExternalOutput")
    d_zs = nc.dram_tensor("zs_o", [1, RPC], dt.float32, kind="ExternalOutput")

    CG = GH // 128   # 4 chunks for global hidden
    CN = NH // 128   # 2 chunks for noise hidden

    with tile.TileContext(nc) as tc:
        with (
            tc.tile_pool(name="wp", bufs=1) as wp,
            tc.tile_pool(name="gp", bufs=2) as gp,
            tc.tile_pool(name="hp", bufs=2 * CG) as hp,
            tc.tile_pool(name="pp", bufs=4, space="PSUM") as pp,
        ):
            # tiny dummy activation with no DMA deps: the table-load
            # pass places the ~1.3us ACT_TABLE_LOAD before it, pulling the
            # load into the idle preamble window off the critical path
            pre = wp.tile([1, 8], dt.bfloat16, name="act_pre")
            nc.vector.memset(pre, 0.0)
            nc.scalar.activation(pre, pre, AFT.Sigmoid)

            # ---- resident loads: noise weights + tile-0 x first so the
            # PE can start on the noise branch while w0g still loads ----
            w0n = wp.tile([F, 3 * NH], dt.bfloat16, name="w0n_sb")
            nc.sync.dma_start(out=w0n, in_=d_w0n[:, :])
            xTs = [wp.tile([F, RT], dt.bfloat16, name=f"xT_sb{t}")
                   for t in range(NT)]
            nc.sync.dma_start(out=xTs[0], in_=d_xT[:, 0:RT])
            bn0 = wp.tile([128, 3 * CN], dt.float32, name="bn0_sb")
            nc.sync.dma_start(out=bn0, in_=d_bn0[:, :])
            w0g = wp.tile([F, 3 * GH], dt.bfloat16, name="w0g_sb")
            nc.sync.dma_start(out=w0g, in_=d_w0g[:, :])
            bg0 = wp.tile([128, 3 * CG], dt.float32, name="bg0_sb")
            nc.sync.dma_start(out=bg0, in_=d_bg0[:, :])
            w1n = [wp.tile([128, 3 * NH], dt.bfloat16, name=f"w1n_sb{k}")
                   for k in range(CN)]
            for k in range(CN):
                nc.sync.dma_start(out=w1n[k], in_=d_w1n[k * 128:(k + 1) * 128, :])
            bn1 = wp.tile([128, 3 * CN], dt.float32, name="bn1_sb")
            nc.sync.dma_start(out=bn1, in_=d_bn1[:, :])
            w1g = [wp.tile([128, 3 * GH], dt.bfloat16, name=f"w1g_sb{k}")
                   for k in range(CG)]
            for k in range(CG):
                nc.sync.dma_start(out=w1g[k], in_=d_w1g[k * 128:(k + 1) * 128, :])
            bg1 = wp.tile([128, 3 * CG], dt.float32, name="bg1_sb")
            nc.sync.dma_start(out=bg1, in_=d_bg1[:, :])
            for t in range(1, NT):
                nc.sync.dma_start(out=xTs[t], in_=d_xT[:, t * RT:(t + 1) * RT])
            wmu = wp.tile([128, CG], dt.bfloat16, name="wmu_sb")
            nc.sync.dma_start(out=wmu, in_=d_wmu[:, :])
            wsig = wp.tile([128, CN], dt.bfloat16, name="wsig_sb")
            nc.sync.dma_start(out=wsig, in_=d_wsig[:, :])

            # PE warmup: dummy accumulating matmuls on zeroed SBUF during
            # the input-DMA window. HAM boots the PE at 1.2 GHz and only
            # releases full clock after ~4us of sustained activity; these
            # burn that ramp while the PE would otherwise idle, so real
            # matmuls start warm. The scratch PSUM is never read and the
            # first real matmul's start=True clears the bank.
            scr = wp.tile([128, HALF], dt.bfloat16, name="warm_scr")
            nc.vector.memset(scr, 0.0)
            pscr = pp.tile([128, RT], dt.float32, tag="ps", bufs=4,
                           name="warm_ps")
            for i in range(24):
                nc.tensor.matmul(pscr[:, 0:HALF], scr[:, 0:128], scr,
                                 start=(i == 0), stop=(i == 23))

            def layer_group(t, C, rhs_list, w_list, b_sb, out_tag, form, relu):
                """One full LSTM step (all C hidden chunks) for RT rows.
                Returns per-chunk emission thunks; no cross-chunk barriers,
                so the PE stream never stalls behind a batched activation.
                rhs_list may contain deferred lists filled by earlier thunks."""
                nk = len(rhs_list)
                hs_out = [None] * C

                def chunk(c):
                    ps = []
                    for gi in range(3):  # i, g, o
                        p = pp.tile([128, RT], dt.float32, tag="ps", bufs=4,
                                    name=f"p_{out_tag}_{t}_{c}_{gi}")
                        mcol = (gi * C + c) * 128
                        for k in range(nk):
                            for h in range(RT // HALF):
                                hs = slice(h * HALF, (h + 1) * HALF)
                                nc.tensor.matmul(
                                    p[:, hs],
                                    w_list[k][:, mcol:mcol + 128],
                                    rhs_list[k][:, hs],
                                    start=(k == 0), stop=(k == nk - 1),
                                )
                        ps.append(p)
                    pi, pg, po = ps
                    ti = gp.tile([128, RT], dt.bfloat16, tag="ti", bufs=4,
                                 name=f"ti_{out_tag}_{t}_{c}")
                    nc.scalar.activation(ti, pi, AFT.Sigmoid, bias=b_sb[:, c:c + 1])
                    tg = gp.tile([128, RT], dt.bfloat16, tag="tg", bufs=4,
                                 name=f"tg_{out_tag}_{t}_{c}")
                    nc.scalar.activation(tg, pg, AFT.Tanh,
                                         bias=b_sb[:, C + c:C + c + 1])
                    to = gp.tile([128, RT], dt.bfloat16, tag="to", bufs=6,
                                 name=f"to_{out_tag}_{t}_{c}")
                    nc.scalar.activation(to, po, AFT.Sigmoid,
                                         bias=b_sb[:, 2 * C + c:2 * C + c + 1])
                    cc = gp.tile([128, RT], dt.bfloat16, tag="cc", bufs=4,
                                 name=f"cc_{out_tag}_{t}_{c}")
                    nc.vector.tensor_mul(cc, ti, tg)
                    th = gp.tile([128, RT], dt.bfloat16, tag="th", bufs=4,
                                 name=f"th_{out_tag}_{t}_{c}")
                    if form == "sq":
                        if relu:
                            # relu(sig(o)*tanh(c)) == sig(o)*tanh(relu(c))
                            nc.vector.tensor_scalar_max(cc, cc, 0.0)
                        tq = gp.tile([128, RT], dt.bfloat16, tag="pta", bufs=3,
                                     name=f"tq_{out_tag}_{t}_{c}")
                        nc.vector.tensor_mul(tq, cc, cc)
                        qq = gp.tile([128, RT], dt.bfloat16, tag="ptb", bufs=3,
                                     name=f"qq_{out_tag}_{t}_{c}")
                        nc.vector.tensor_scalar(qq, tq, PD1, PD0, op0=ALU.mult,
                                                op1=ALU.add)
                        nc.vector.tensor_mul(th, qq, cc)
                    else:
                        # layer-1 poly: th = (A0 + A1*c)*c, one fused
                        # tensor_scalar + one tensor_tensor. No max needed:
                        # for c in [-1,0), A0+A1*c > 0 so sign(h) == sign(c)
                        # and the final relu on h zeroes it, matching
                        # tanh(relu(c)) == 0; for c >= 0 it is the fitted
                        # c*(A0+A1*c) quad exactly.
                        a0, a1 = form
                        w = gp.tile([128, RT], dt.bfloat16, tag="ptb", bufs=3,
                                    name=f"w_{out_tag}_{t}_{c}")
                        nc.vector.tensor_scalar(w, cc, a1, a0,
                                                op0=ALU.mult, op1=ALU.add)
                        nc.vector.tensor_mul(th, w, cc)
                    h = hp.tile([128, RT], dt.bfloat16, tag=out_tag,
                                bufs=(3 if relu else 2) * C,
                                name=f"h_{out_tag}_{t}_{c}")
                    nc.vector.tensor_mul(h, to, th)
                    if form != "sq" and relu:
                        nc.vector.tensor_scalar_max(h, h, 0.0)
                    hs_out[c] = h

                thunks = [partial(chunk, c) for c in range(C)]
                return thunks, hs_out

            def tail_thunks(t, r1g, r1n, last=False):
                # single-row sums: mu[row] = wmu . r1g[:, row] at partition 0,
                # sig[row] = wsig . r1n[:, row] at partition 32 of the same
                # PSUM tile -> one ACT staging copy serves both outputs.
                box = {}

                def emit_mu():
                    pz = pp.tile([33, RT], dt.float32, tag="ps", bufs=4,
                                 name=f"pz_{t}")
                    box["pz"] = pz
                    for k in range(CG):
                        for h in range(RT // HALF):
                            hs = slice(h * HALF, (h + 1) * HALF)
                            nc.tensor.matmul(pz[0:1, hs], wmu[:, k:k + 1],
                                             r1g[k][:, hs],
                                             start=(k == 0), stop=(k == CG - 1))
                    if last:
                        stm = gp.tile([1, RT], dt.float32, tag="stm", bufs=1,
                                      name=f"stm_{t}")
                        nc.scalar.copy(stm, pz[0:1, :])
                        nc.sync.dma_start(out=d_mus[:, t * RT:(t + 1) * RT],
                                          in_=stm)

                def emit_sig():
                    pz = box["pz"]
                    for k in range(CN):
                        for h in range(RT // HALF):
                            hs = slice(h * HALF, (h + 1) * HALF)
                            nc.tensor.matmul(pz[32:33, hs], wsig[:, k:k + 1],
                                             r1n[k][:, hs],
                                             start=(k == 0), stop=(k == CN - 1),
                                             tile_position=(0, 32))
                    if last:
                        sts = gp.tile([1, RT], dt.float32, tag="sts", bufs=1,
                                      name=f"sts_{t}")
                        nc.scalar.copy(sts, pz[32:33, :])
                        nc.sync.dma_start(out=d_zs[:, t * RT:(t + 1) * RT],
                                          in_=sts)
                    else:
                        st = gp.tile([33, RT], dt.float32, tag="st", bufs=2,
                                     name=f"st_{t}")
                        # lands in ACT's idle window (PE-heavy phase), off DVE
                        nc.scalar.copy(st, pz)
                        nc.sync.dma_start(out=d_mus[:, t * RT:(t + 1) * RT],
                                          in_=st[0:1, :])
                        nc.sync.dma_start(out=d_zs[:, t * RT:(t + 1) * RT],
                                          in_=st[32:33, :])

                return emit_mu, emit_sig

            # Software pipeline with one-tile skew: tile t's layer-0 work
            # (ACT-heavy, PE-light) is emitted interleaved with tile t-1's
            # layer-1 work (PE-heavy, ACT-light), so both engine queues stay
            # dense and the PE never idles long enough to lose HAM warmth.
            light, heavy, tails = [], [], []
            for t in range(NT):
                b_th, h0n = layer_group(t, CN, [xTs[t]], [w0n], bn0, "h0n",
                                        "sq", False)
                a_th, h0g = layer_group(t, CG, [xTs[t]], [w0g], bg0, "h0g",
                                        "sq", False)
                c_th, r1g = layer_group(t, CG, h0g, w1g, bg1, "r1g",
                                        ABg, True)
                d_th, r1n = layer_group(t, CN, h0n, w1n, bn1, "r1n",
                                        ABn, True)
                mu_th, sig_th = tail_thunks(t, r1g, r1n, last=t == NT - 1)
                # tile 0 runs standalone: noise first so the PE starts while
                # w0g still loads; later tiles interleave against the
                # PE-heavy layer-1 stream, where global L0 (ACT-heavy) must
                # lead to keep both queues dense
                light.append(b_th + a_th if t == 0 else a_th + b_th)
                heavy.append(c_th + d_th)
                tails.append([mu_th, sig_th])

            def interleave(xs, ys):
                out = []
                n = max(len(xs), len(ys))
                for i in range(n):
                    if i < len(xs):
                        out.append(xs[i])
                    if i < len(ys):
                        out.append(ys[i])
                return out

            # mu/sig tails have no consumers, so they are emitted a full
            # round after their r1 inputs: their matmuls are always
            # instantly ready and never head-of-line-block the PE FIFO
            for th in light[0]:
                th()
            for r in range(1, NT):
                stream = heavy[r - 1] + (tails[r - 2] if r >= 2 else [])
                for th in interleave(stream, light[r]):
                    th()
            # final drain: r1g chunks first, then r1n with the tails slotted
            # as soon as their inputs are ready
            fin = heavy[NT - 1]
            for th in fin[:CG]:
                th()
            tails[NT - 2][0]()        # old mu (inputs long ready)
            fin[CG]()                 # r1n chunk 0
            tails[NT - 2][1]()        # old sig + copy + dma
            fin[CG + 1]()             # r1n chunk 1
            tails[NT - 1][0]()        # this tile's mu (r1g ready)
            tails[NT - 1][1]()        # final sig + copy + dma

    nc.compile()
    return nc


def _pack_lstm_weights(W, b, H):
    """Drop the f gate; pack [i, g, o] along the output dim.
    Returns lhsT (K, 3H) bf16 and bias tile (128, 3H/128) f32."""
    idx = np.r_[0:H, 2 * H:3 * H, 3 * H:4 * H]
    Wp = W[idx]                      # (3H, K)
    bp = b[idx]                      # (3H,)
    lhsT = np.ascontiguousarray(Wp.T).astype(BF16)
    btile = np.ascontiguousarray(bp.reshape(3 * H // 128, 128).T).astype(np.float32)
    return lhsT, btile


def _make_in_maps(inputs):
    """Host-side packing: shard X, drop f-gates, fold aff into one dot.
    Returns (per-core input maps, summed aff bias, noise bias)."""
    X = np.asarray(inputs["X"], np.float32)
    g_Wih0 = np.asarray(inputs["g_Wih0"], np.float32)
    g_b0 = np.asarray(inputs["g_b0"], np.float32)
    g_Wih1 = np.asarray(inputs["g_Wih1"], np.float32)
    g_b1 = np.asarray(inputs["g_b1"], np.float32)
    aff_W = np.asarray(inputs["aff_W"], np.float32)
    aff_b = np.asarray(inputs["aff_b"], np.float32)
    n_Wih0 = np.asarray(inputs["n_Wih0"], np.float32)
    n_b0 = np.asarray(inputs["n_b0"], np.float32)
    n_Wih1 = np.asarray(inputs["n_Wih1"], np.float32)
    n_b1 = np.asarray(inputs["n_b1"], np.float32)
    noise_W = np.asarray(inputs["noise_W"], np.float32)
    noise_b = np.asarray(inputs["noise_b"], np.float32)

    w0g, bg0 = _pack_lstm_weights(g_Wih0, g_b0, GH)
    w1g, bg1 = _pack_lstm_weights(g_Wih1, g_b1, GH)
    w0n, bn0 = _pack_lstm_weights(n_Wih0, n_b0, NH)
    w1n, bn1 = _pack_lstm_weights(n_Wih1, n_b1, NH)

    wm = aff_W.sum(axis=0)                     # (GH,)
    wmu = np.ascontiguousarray(wm.reshape(GH // 128, 128).T).astype(BF16)
    b_mu = float(aff_b.sum())
    ws = noise_W[0]                            # (NH,)
    wsig = np.ascontiguousarray(ws.reshape(NH // 128, 128).T).astype(BF16)
    b_sig = float(noise_b[0])

    Xf = X.reshape(ROWS, F)
    shared = {
        "w0g": w0g, "w1g": w1g, "w0n": w0n, "w1n": w1n,
        "wmu": wmu, "wsig": wsig,
        "bg0": bg0, "bg1": bg1, "bn0": bn0, "bn1": bn1,
    }
    in_maps = []
    for c in range(NCORES):
        xc = np.ascontiguousarray(
            Xf[c * RPC:(c + 1) * RPC].T).astype(BF16)    # (F, RPC)
        in_maps.append({"xT": xc, **shared})
    return in_maps, b_mu, b_sig


def kernel(**inputs):
    from concourse.bass_utils import run_bass_kernel_spmd

    in_maps, b_mu, b_sig = _make_in_maps(inputs)
    if "nc" not in _CACHE:
        _CACHE["nc"] = _build_program()
    nc = _CACHE["nc"]

    res = run_bass_kernel_spmd(nc, in_maps, list(range(NCORES)))

    mus = np.empty(ROWS, np.float32)
    zs = np.empty(ROWS, np.float32)
    for c in range(NCORES):
        mus[c * RPC:(c + 1) * RPC] = res.results[c]["mus_o"][0]
        zs[c * RPC:(c + 1) * RPC] = res.results[c]["zs_o"][0]
    # device outputs the raw row sums; the constant aff bias, the softplus
    # epilogue over 32k scalars, and the +1e-6 epsilon fold on host
    mus = (mus + b_mu).reshape(NTS, NPER)
    sig = (np.logaddexp(0.0, zs + b_sig).astype(np.float32) + 1e-6).reshape(NTS, NPER)
    return mus, sig


# revision 25
# speedup vs baseline: 1.0201x; 1.0201x over previous
"""DeepFactorRNN Trainium2 kernel.

Computes, for x = X.reshape(-1, F):
  mus    = sum_j(relu(LSTM2g(LSTM1g(x))) @ aff_W.T + aff_b)_j
  sigmas = softplus(relu(LSTM2n(LSTM1n(x))) @ noise_W.T + noise_b) + 1e-6
where each LSTM is a single step from zero state (so the forget gate is
unused and c = sigmoid(i)*tanh(g), h = sigmoid(o)*tanh(c)).

Strategy (8 NeuronCores, data parallel over the 32768 flattened rows):
 - Rows live on the matmul free dim; features/gates on partitions, so the
   whole network is transpose-free. X is transposed/cast on host.
 - f-gates are dropped from all weight matrices (25% matmul savings).
 - The aff linear + sum collapses to one dot with w_mu = aff_W.sum(0).
 - bf16 matmul operands, fp32 PSUM accumulation, fp32 activation math.
 - ACT keeps only the 3 gate table passes per chunk (its per-element
   floor); tanh(c) runs on the DVE (|c|<=1) as least-squares polys fitted
   to the empirical c distribution:
     layer 0:  c*(PD0 + PD1*c^2)  2 tensor_tensor + 1 tensor_scalar
     layer 1:  c*(A0  + A1*c)     1 tensor_tensor + 1 fused tensor_scalar
   The layer-1 quad needs no max/relu stage: on [-1,0) A0+A1*c > 0 keeps
   sign(h) == sign(c), so the single final relu on h reproduces
   tanh(relu(c)) exactly while fitting the c>=0 half-range better than an
   odd cubic. scalar_tensor_tensor is never used (it runs at DVE 1x mode;
   tensor_scalar runs 4x, tensor_tensor 2x).
 - Work stays at single-chunk granularity ([128, RT] tiles): wider fused
   tiles serialize the engines through the 8-bank PSUM, and measurably
   denser engine overlap trips a chip-wide ~1.2x clock throttle (observed
   deterministically per compiled NEFF), so instruction savings are taken
   only where they do not tighten simultaneous engine activity further.
 - Emission is software-pipelined with a one-tile skew: tile t layer-0
   chunks (ACT-heavy) interleave with tile t-1 layer-1 chunks (PE-heavy)
   so all engine queues stay dense.
 - Tail row-sums for mu and sigma share one PSUM tile (sigma lands at
   partition 32 via tile_position) so one ACT staging copy serves both,
   landing in ACT idle windows; the last tile splits the copies so the mu
   DMA overlaps the remaining noise compute during the drain.
 - A dummy activation with no DMA deps hoists the ~1.3us ACT table load
   into the preamble; the noise branch is computed first at startup (its
   weights are 4x smaller) so the PE starts while w0g still loads; tails
   are emitted a full round late so their matmuls never head-of-line
   block the PE queue.
 - The constant aff bias and the softplus epilogue fold on the host.
"""

from functools import partial

import numpy as np
import ml_dtypes

BF16 = ml_dtypes.bfloat16

NCORES = 8
NTS, NPER, F = 128, 256, 128
GH, NH = 512, 256
ROWS = NTS * NPER            # 32768
RPC = ROWS // NCORES         # 4096 rows per core
RT = 1024                    # rows per tile
NT = RPC // RT               # 4 tiles per core
HALF = 512                   # matmul moving max: one PSUM bank of fp32

# tanh(x) ~= x*(PD0 + PD1*x^2), least-squares fit over the empirical
# c = sigmoid(i)*tanh(g) distribution of all four LSTM layers
PD0, PD1 = 0.9925682, -0.26160714
# tanh(x) ~= x*(A0 + A1*x) for x >= 0, fit per-branch over the relu'd
# layer-1 c distribution (used with the relu folded into the max op;
# on the c>=0 half-range this quad fit beats the odd cubic)
ABg = (1.0057485, -0.09441389)
ABn = (1.0060241, -0.09650258)

_CACHE = {}


def _build_program():
    import concourse.bacc as bacc
    import concourse.tile as tile
    from concourse import mybir

    dt = mybir.dt
    AFT = mybir.ActivationFunctionType
    ALU = mybir.AluOpType

    nc = bacc.Bacc("TRN2", target_bir_lowering=False, debug=False,
                   num_devices=NCORES)

    # ---- DRAM I/O ----
    d_xT = nc.dram_tensor("xT", [F, RPC], dt.bfloat16, kind="ExternalInput")
    d_w0g = nc.dram_tensor("w0g", [F, 3 * GH], dt.bfloat16, kind="ExternalInput")
    d_w1g = nc.dram_tensor("w1g", [GH, 3 * GH], dt.bfloat16, kind="ExternalInput")
    d_w0n = nc.dram_tensor("w0n", [F, 3 * NH], dt.bfloat16, kind="ExternalInput")
    d_w1n = nc.dram_tensor("w1n", [NH, 3 * NH], dt.bfloat16, kind="ExternalInput")
    d_wmu = nc.dram_tensor("wmu", [128, GH // 128], dt.bfloat16, kind="ExternalInput")
    d_wsig = nc.dram_tensor("wsig", [128, NH // 128], dt.bfloat16, kind="ExternalInput")
    d_bg0 = nc.dram_tensor("bg0", [128, 3 * GH // 128], dt.float32, kind="ExternalInput")
    d_bg1 = nc.dram_tensor("bg1", [128, 3 * GH // 128], dt.float32, kind="ExternalInput")
    d_bn0 = nc.dram_tensor("bn0", [128, 3 * NH // 128], dt.float32, kind="ExternalInput")
    d_bn1 = nc.dram_tensor("bn1", [128, 3 * NH // 128], dt.float32, kind="ExternalInput")
    d_mus = nc.dram_tensor("mus_o", [1, RPC], dt.float32, kind="ExternalOutput")
    d_zs = nc.dram_tensor("zs_o", [1, RPC], dt.float32, kind="ExternalOutput")

    CG = GH // 128   # 4 chunks for global hidden
    CN = NH // 128   # 2 chunks for noise hidden

    with tile.TileContext(nc) as tc:
        with (
            tc.tile_pool(name="wp", bufs=1) as wp,
            tc.tile_pool(name="gp", bufs=2) as gp,
            tc.tile_pool(name="hp", bufs=2 * CG) as hp,
            tc.tile_pool(name="pp", bufs=4, space="PSUM") as pp,
        ):
            # tiny dummy activation with no DMA deps: the table-load
            # pass places the ~1.3us ACT_TABLE_LOAD before it, pulling the
            # load into the idle preamble window off the critical path
            pre = wp.tile([1, 8], dt.bfloat16, name="act_pre")
            nc.vector.memset(pre, 0.0)
            nc.scalar.activation(pre, pre, AFT.Sigmoid)

            # ---- resident loads: noise weights + tile-0 x first so the
            # PE can start on the noise branch while w0g still loads ----
            w0n = wp.tile([F, 3 * NH], dt.bfloat16, name="w0n_sb")
            nc.sync.dma_start(out=w0n, in_=d_w0n[:, :])
            xTs = [wp.tile([F, RT], dt.bfloat16, name=f"xT_sb{t}")
                   for t in range(NT)]
            nc.sync.dma_start(out=xTs[0], in_=d_xT[:, 0:RT])
            bn0 = wp.tile([128, 3 * CN], dt.float32, name="bn0_sb")
            nc.sync.dma_start(out=bn0, in_=d_bn0[:, :])
            w0g = wp.tile([F, 3 * GH], dt.bfloat16, name="w0g_sb")
            nc.sync.dma_start(out=w0g, in_=d_w0g[:, :])
            bg0 = wp.tile([128, 3 * CG], dt.float32, name="bg0_sb")
            nc.sync.dma_start(out=bg0, in_=d_bg0[:, :])
            w1n = [wp.tile([128, 3 * NH], dt.bfloat16, name=f"w1n_sb{k}")
                   for k in range(CN)]
            for k in range(CN):
                nc.sync.dma_start(out=w1n[k], in_=d_w1n[k * 128:(k + 1) * 128, :])
            bn1 = wp.tile([128, 3 * CN], dt.float32, name="bn1_sb")
            nc.sync.dma_start(out=bn1, in_=d_bn1[:, :])
            w1g = [wp.tile([128, 3 * GH], dt.bfloat16, name=f"w1g_sb{k}")
                   for k in range(CG)]
            for k in range(CG):
                nc.sync.dma_start(out=w1g[k], in_=d_w1g[k * 128:(k + 1) * 128, :])
            bg1 = wp.tile([128, 3 * CG], dt.float32, name="bg1_sb")
            nc.sync.dma_start(out=bg1, in_=d_bg1[:, :])
            for t in range(1, NT):
                nc.sync.dma_start(out=xTs[t], in_=d_xT[:, t * RT:(t + 1) * RT])
            wmu = wp.tile([128, CG], dt.bfloat16, name="wmu_sb")
            nc.sync.dma_start(out=wmu, in_=d_wmu[:, :])
            wsig = wp.tile([128, CN], dt.bfloat16, name="wsig_sb")
            nc.sync.dma_start(out=wsig, in_=d_wsig[:, :])

            def layer_group(t, C, rhs_list, w_list, b_sb, out_tag, form, relu):
                """One full LSTM step (all C hidden chunks) for RT rows.
                Returns per-chunk emission thunks; no cross-chunk barriers,
                so the PE stream never stalls behind a batched activation.
                rhs_list may contain deferred lists filled by earlier thunks."""
                nk = len(rhs_list)
                hs_out = [None] * C

                def chunk(c):
                    ps = []
                    for gi in range(3):  # i, g, o
                        p = pp.tile([128, RT], dt.float32, tag="ps", bufs=4,
                                    name=f"p_{out_tag}_{t}_{c}_{gi}")
                        mcol = (gi * C + c) * 128
                        for k in range(nk):
                            for h in range(RT // HALF):
                                hs = slice(h * HALF, (h + 1) * HALF)
                                nc.tensor.matmul(
                                    p[:, hs],
                                    w_list[k][:, mcol:mcol + 128],
                                    rhs_list[k][:, hs],
                                    start=(k == 0), stop=(k == nk - 1),
                                )
                        ps.append(p)
                    pi, pg, po = ps
                    ti = gp.tile([128, RT], dt.bfloat16, tag="ti", bufs=5,
                                 name=f"ti_{out_tag}_{t}_{c}")
                    nc.scalar.activation(ti, pi, AFT.Sigmoid, bias=b_sb[:, c:c + 1])
                    tg = gp.tile([128, RT], dt.bfloat16, tag="tg", bufs=5,
                                 name=f"tg_{out_tag}_{t}_{c}")
                    nc.scalar.activation(tg, pg, AFT.Tanh,
                                         bias=b_sb[:, C + c:C + c + 1])
                    to = gp.tile([128, RT], dt.bfloat16, tag="to", bufs=7,
                                 name=f"to_{out_tag}_{t}_{c}")
                    nc.scalar.activation(to, po, AFT.Sigmoid,
                                         bias=b_sb[:, 2 * C + c:2 * C + c + 1])
                    cc = gp.tile([128, RT], dt.bfloat16, tag="cc", bufs=4,
                                 name=f"cc_{out_tag}_{t}_{c}")
                    nc.vector.tensor_mul(cc, ti, tg)
                    th = gp.tile([128, RT], dt.bfloat16, tag="th", bufs=4,
                                 name=f"th_{out_tag}_{t}_{c}")
                    if form == "sq":
                        if relu:
                            # relu(sig(o)*tanh(c)) == sig(o)*tanh(relu(c))
                            nc.vector.tensor_scalar_max(cc, cc, 0.0)
                        tq = gp.tile([128, RT], dt.bfloat16, tag="pta", bufs=3,
                                     name=f"tq_{out_tag}_{t}_{c}")
                        nc.vector.tensor_mul(tq, cc, cc)
                        qq = gp.tile([128, RT], dt.bfloat16, tag="ptb", bufs=3,
                                     name=f"qq_{out_tag}_{t}_{c}")
                        nc.vector.tensor_scalar(qq, tq, PD1, PD0, op0=ALU.mult,
                                                op1=ALU.add)
                        nc.vector.tensor_mul(th, qq, cc)
                    else:
                        # layer-1 poly: th = (A0 + A1*c)*c, one fused
                        # tensor_scalar + one tensor_tensor. No max needed:
                        # for c in [-1,0), A0+A1*c > 0 so sign(h) == sign(c)
                        # and the final relu on h zeroes it, matching
                        # tanh(relu(c)) == 0; for c >= 0 it is the fitted
                        # c*(A0+A1*c) quad exactly.
                        a0, a1 = form
                        w = gp.tile([128, RT], dt.bfloat16, tag="ptb", bufs=3,
                                    name=f"w_{out_tag}_{t}_{c}")
                        nc.vector.tensor_scalar(w, cc, a1, a0,
                                                op0=ALU.mult, op1=ALU.add)
                        nc.vector.tensor_mul(th, w, cc)
                    h = hp.tile([128, RT], dt.bfloat16, tag=out_tag,
                                bufs=(3 if relu else 2) * C,
                                name=f"h_{out_tag}_{t}_{c}")
                    nc.vector.tensor_mul(h, to, th)
                    if form != "sq" and relu:
                        nc.vector.tensor_scalar_max(h, h, 0.0)
                    hs_out[c] = h

                thunks = [partial(chunk, c) for c in range(C)]
                return thunks, hs_out

            def tail_thunks(t, r1g, r1n, last=False):
                # single-row sums: mu[row] = wmu . r1g[:, row] at partition 0,
                # sig[row] = wsig . r1n[:, row] at partition 32 of the same
                # PSUM tile -> one ACT staging copy serves both outputs.
                box = {}

                def emit_mu():
                    pz = pp.tile([33, RT], dt.float32, tag="ps", bufs=4,
                                 name=f"pz_{t}")
                    box["pz"] = pz
                    for k in range(CG):
                        for h in range(RT // HALF):
                            hs = slice(h * HALF, (h + 1) * HALF)
                            nc.tensor.matmul(pz[0:1, hs], wmu[:, k:k + 1],
                                             r1g[k][:, hs],
                                             start=(k == 0), stop=(k == CG - 1))
                    if last:
                        stm = gp.tile([1, RT], dt.float32, tag="stm", bufs=1,
                                      name=f"stm_{t}")
                        nc.scalar.copy(stm, pz[0:1, :])
                        nc.sync.dma_start(out=d_mus[:, t * RT:(t + 1) * RT],
                                          in_=stm)

                def emit_sig():
                    pz = box["pz"]
                    for k in range(CN):
                        for h in range(RT // HALF):
                            hs = slice(h * HALF, (h + 1) * HALF)
                            nc.tensor.matmul(pz[32:33, hs], wsig[:, k:k + 1],
                                             r1n[k][:, hs],
                                             start=(k == 0), stop=(k == CN - 1),
                                             tile_position=(0, 32))
                    if last:
                        sts = gp.tile([1, RT], dt.float32, tag="sts", bufs=1,
                                      name=f"sts_{t}")
                        nc.scalar.copy(sts, pz[32:33, :])
                        nc.sync.dma_start(out=d_zs[:, t * RT:(t + 1) * RT],
                                          in_=sts)
                    else:
                        st = gp.tile([33, RT], dt.float32, tag="st", bufs=2,
                                     name=f"st_{t}")
                        # on the DVE: ACT and PE co-bind, the DVE has slack
                        nc.vector.tensor_copy(st, pz)
                        nc.sync.dma_start(out=d_mus[:, t * RT:(t + 1) * RT],
                                          in_=st[0:1, :])
                        nc.sync.dma_start(out=d_zs[:, t * RT:(t + 1) * RT],
                                          in_=st[32:33, :])

                return emit_mu, emit_sig

            # Software pipeline with one-tile skew: tile t's layer-0 work
            # (ACT-heavy, PE-light) is emitted interleaved with tile t-1's
            # layer-1 work (PE-heavy, ACT-light), so both engine queues stay
            # dense and the PE never idles long enough to lose HAM warmth.
            light, heavy, tails = [], [], []
            for t in range(NT):
                b_th, h0n = layer_group(t, CN, [xTs[t]], [w0n], bn0, "h0n",
                                        "sq", False)
                a_th, h0g = layer_group(t, CG, [xTs[t]], [w0g], bg0, "h0g",
                                        "sq", False)
                c_th, r1g = layer_group(t, CG, h0g, w1g, bg1, "r1g",
                                        ABg, True)
                d_th, r1n = layer_group(t, CN, h0n, w1n, bn1, "r1n",
                                        ABn, True)
                mu_th, sig_th = tail_thunks(t, r1g, r1n, last=t == NT - 1)
                # tile 0 runs standalone: noise first so the PE starts while
                # w0g still loads; later tiles interleave against the
                # PE-heavy layer-1 stream, where global L0 (ACT-heavy) must
                # lead to keep both queues dense
                light.append(b_th + a_th if t == 0 else a_th + b_th)
                heavy.append(c_th + d_th)
                tails.append([mu_th, sig_th])

            def interleave(xs, ys):
                out = []
                n = max(len(xs), len(ys))
                for i in range(n):
                    if i < len(xs):
                        out.append(xs[i])
                    if i < len(ys):
                        out.append(ys[i])
                return out

            # mu/sig tails have no consumers, so they are emitted a full
            # round after their r1 inputs: their matmuls are always
            # instantly ready and never head-of-line-block the PE FIFO
            for th in light[0]:
                th()
            for r in range(1, NT):
                stream = heavy[r - 1] + (tails[r - 2] if r >= 2 else [])
                for th in interleave(stream, light[r]):
                    th()
            # final drain: r1g chunks first, then r1n with the tails slotted
            # as soon as their inputs are ready
            fin = heavy[NT - 1]
            for th in fin[:CG]:
                th()
            tails[NT - 2][0]()        # old mu (inputs long ready)
            fin[CG]()                 # r1n chunk 0
            tails[NT - 2][1]()        # old sig + copy + dma
            fin[CG + 1]()             # r1n chunk 1
            tails[NT - 1][0]()        # this tile's mu (r1g ready)
            tails[NT - 1][1]()        # final sig + copy + dma

    nc.compile()
    return nc


def _pack_lstm_weights(W, b, H):
    """Drop the f gate; pack [i, g, o] along the output dim.
    Returns lhsT (K, 3H) bf16 and bias tile (128, 3H/128) f32."""
    idx = np.r_[0:H, 2 * H:3 * H, 3 * H:4 * H]
    Wp = W[idx]                      # (3H, K)
    bp = b[idx]                      # (3H,)
    lhsT = np.ascontiguousarray(Wp.T).astype(BF16)
    btile = np.ascontiguousarray(bp.reshape(3 * H // 128, 128).T).astype(np.float32)
    return lhsT, btile


def _make_in_maps(inputs):
    """Host-side packing: shard X, drop f-gates, fold aff into one dot.
    Returns (per-core input maps, summed aff bias, noise bias)."""
    X = np.asarray(inputs["X"], np.float32)
    g_Wih0 = np.asarray(inputs["g_Wih0"], np.float32)
    g_b0 = np.asarray(inputs["g_b0"], np.float32)
    g_Wih1 = np.asarray(inputs["g_Wih1"], np.float32)
    g_b1 = np.asarray(inputs["g_b1"], np.float32)
    aff_W = np.asarray(inputs["aff_W"], np.float32)
    aff_b = np.asarray(inputs["aff_b"], np.float32)
    n_Wih0 = np.asarray(inputs["n_Wih0"], np.float32)
    n_b0 = np.asarray(inputs["n_b0"], np.float32)
    n_Wih1 = np.asarray(inputs["n_Wih1"], np.float32)
    n_b1 = np.asarray(inputs["n_b1"], np.float32)
    noise_W = np.asarray(inputs["noise_W"], np.float32)
    noise_b = np.asarray(inputs["noise_b"], np.float32)

    w0g, bg0 = _pack_lstm_weights(g_Wih0, g_b0, GH)
    w1g, bg1 = _pack_lstm_weights(g_Wih1, g_b1, GH)
    w0n, bn0 = _pack_lstm_weights(n_Wih0, n_b0, NH)
    w1n, bn1 = _pack_lstm_weights(n_Wih1, n_b1, NH)

    wm = aff_W.sum(axis=0)                     # (GH,)
    wmu = np.ascontiguousarray(wm.reshape(GH // 128, 128).T).astype(BF16)
    b_mu = float(aff_b.sum())
    ws = noise_W[0]                            # (NH,)
    wsig = np.ascontiguousarray(ws.reshape(NH // 128, 128).T).astype(BF16)
    b_sig = float(noise_b[0])

    Xf = X.reshape(ROWS, F)
    shared = {
        "w0g": w0g, "w1g": w1g, "w0n": w0n, "w1n": w1n,
        "wmu": wmu, "wsig": wsig,
        "bg0": bg0, "bg1": bg1, "bn0": bn0, "bn1": bn1,
    }
    in_maps = []
    for c in range(NCORES):
        xc = np.ascontiguousarray(
            Xf[c * RPC:(c + 1) * RPC].T).astype(BF16)    # (F, RPC)
        in_maps.append({"xT": xc, **shared})
    return in_maps, b_mu, b_sig


def kernel(**inputs):
    from concourse.bass_utils import run_bass_kernel_spmd

    in_maps, b_mu, b_sig = _make_in_maps(inputs)
    if "nc" not in _CACHE:
        _CACHE["nc"] = _build_program()
    nc = _CACHE["nc"]

    res = run_bass_kernel_spmd(nc, in_maps, list(range(NCORES)))

    mus = np.empty(ROWS, np.float32)
    zs = np.empty(ROWS, np.float32)
    for c in range(NCORES):
        mus[c * RPC:(c + 1) * RPC] = res.results[c]["mus_o"][0]
        zs[c * RPC:(c + 1) * RPC] = res.results[c]["zs_o"][0]
    # device outputs the raw row sums; the constant aff bias, the softplus
    # epilogue over 32k scalars, and the +1e-6 epsilon fold on host
    mus = (mus + b_mu).reshape(NTS, NPER)
    sig = (np.logaddexp(0.0, zs + b_sig).astype(np.float32) + 1e-6).reshape(NTS, NPER)
    return mus, sig


# revision 26
# speedup vs baseline: 1.0270x; 1.0068x over previous
"""DeepFactorRNN Trainium2 kernel.

Computes, for x = X.reshape(-1, F):
  mus    = sum_j(relu(LSTM2g(LSTM1g(x))) @ aff_W.T + aff_b)_j
  sigmas = softplus(relu(LSTM2n(LSTM1n(x))) @ noise_W.T + noise_b) + 1e-6
where each LSTM is a single step from zero state (so the forget gate is
unused and c = sigmoid(i)*tanh(g), h = sigmoid(o)*tanh(c)).

Strategy (8 NeuronCores, data parallel over the 32768 flattened rows):
 - Rows live on the matmul free dim; features/gates on partitions, so the
   whole network is transpose-free. X is transposed/cast on host.
 - f-gates are dropped from all weight matrices (25% matmul savings).
 - The aff linear + sum collapses to one dot with w_mu = aff_W.sum(0).
 - bf16 matmul operands, fp32 PSUM accumulation, fp32 activation math.
 - ACT keeps only the 3 gate table passes per chunk (its per-element
   floor); tanh(c) runs on the DVE (|c|<=1) as least-squares polys fitted
   to the empirical c distribution:
     layer 0:  c*(PD0 + PD1*c^2)  2 tensor_tensor + 1 tensor_scalar
     layer 1:  c*(A0  + A1*c)     1 tensor_tensor + 1 fused tensor_scalar
   The layer-1 quad needs no max/relu stage: on [-1,0) A0+A1*c > 0 keeps
   sign(h) == sign(c), so the single final relu on h reproduces
   tanh(relu(c)) exactly while fitting the c>=0 half-range better than an
   odd cubic. scalar_tensor_tensor is never used (it runs at DVE 1x mode;
   tensor_scalar runs 4x, tensor_tensor 2x).
 - Work stays at single-chunk granularity ([128, RT] tiles): wider fused
   tiles serialize the engines through the 8-bank PSUM, and measurably
   denser engine overlap trips a chip-wide ~1.2x clock throttle (observed
   deterministically per compiled NEFF), so instruction savings are taken
   only where they do not tighten simultaneous engine activity further.
 - Emission is software-pipelined with a one-tile skew: tile t layer-0
   chunks (ACT-heavy) interleave with tile t-1 layer-1 chunks (PE-heavy)
   so all engine queues stay dense.
 - Tail row-sums for mu and sigma share one PSUM tile (sigma lands at
   partition 32 via tile_position) so one ACT staging copy serves both,
   landing in ACT idle windows; the last tile splits the copies so the mu
   DMA overlaps the remaining noise compute during the drain.
 - A dummy activation with no DMA deps hoists the ~1.3us ACT table load
   into the preamble; the noise branch is computed first at startup (its
   weights are 4x smaller) so the PE starts while w0g still loads; tails
   are emitted a full round late so their matmuls never head-of-line
   block the PE queue.
 - The constant aff bias and the softplus epilogue fold on the host.
"""

from functools import partial

import numpy as np
import ml_dtypes

BF16 = ml_dtypes.bfloat16

NCORES = 8
NTS, NPER, F = 128, 256, 128
GH, NH = 512, 256
ROWS = NTS * NPER            # 32768
RPC = ROWS // NCORES         # 4096 rows per core
RT = 1024                    # rows per tile
NT = RPC // RT               # 4 tiles per core
HALF = 512                   # matmul moving max: one PSUM bank of fp32

# tanh(x) ~= x*(PD0 + PD1*x^2), least-squares fit over the empirical
# c = sigmoid(i)*tanh(g) distribution of all four LSTM layers
PD0, PD1 = 0.9925682, -0.26160714
# tanh(x) ~= x*(A0 + A1*x) for x >= 0, fit per-branch over the relu'd
# layer-1 c distribution (used with the relu folded into the max op;
# on the c>=0 half-range this quad fit beats the odd cubic)
ABg = (1.0057485, -0.09441389)
ABn = (1.0060241, -0.09650258)

_CACHE = {}


def _build_program():
    import concourse.bacc as bacc
    import concourse.tile as tile
    from concourse import mybir

    dt = mybir.dt
    AFT = mybir.ActivationFunctionType
    ALU = mybir.AluOpType

    nc = bacc.Bacc("TRN2", target_bir_lowering=False, debug=False,
                   num_devices=NCORES)

    # ---- DRAM I/O ----
    d_xT = nc.dram_tensor("xT", [F, RPC], dt.bfloat16, kind="ExternalInput")
    d_w0g = nc.dram_tensor("w0g", [F, 3 * GH], dt.bfloat16, kind="ExternalInput")
    d_w1g = nc.dram_tensor("w1g", [GH, 3 * GH], dt.bfloat16, kind="ExternalInput")
    d_w0n = nc.dram_tensor("w0n", [F, 3 * NH], dt.bfloat16, kind="ExternalInput")
    d_w1n = nc.dram_tensor("w1n", [NH, 3 * NH], dt.bfloat16, kind="ExternalInput")
    d_wmu = nc.dram_tensor("wmu", [128, GH // 128], dt.bfloat16, kind="ExternalInput")
    d_wsig = nc.dram_tensor("wsig", [128, NH // 128], dt.bfloat16, kind="ExternalInput")
    d_bg0 = nc.dram_tensor("bg0", [128, 3 * GH // 128], dt.float32, kind="ExternalInput")
    d_bg1 = nc.dram_tensor("bg1", [128, 3 * GH // 128], dt.float32, kind="ExternalInput")
    d_bn0 = nc.dram_tensor("bn0", [128, 3 * NH // 128], dt.float32, kind="ExternalInput")
    d_bn1 = nc.dram_tensor("bn1", [128, 3 * NH // 128], dt.float32, kind="ExternalInput")
    d_mus = nc.dram_tensor("mus_o", [1, RPC], dt.float32, kind="ExternalOutput")
    d_zs = nc.dram_tensor("zs_o", [1, RPC], dt.float32, kind="ExternalOutput")

    CG = GH // 128   # 4 chunks for global hidden
    CN = NH // 128   # 2 chunks for noise hidden

    with tile.TileContext(nc) as tc:
        with (
            tc.tile_pool(name="wp", bufs=1) as wp,
            tc.tile_pool(name="gp", bufs=2) as gp,
            tc.tile_pool(name="hp", bufs=2 * CG) as hp,
            tc.tile_pool(name="pp", bufs=4, space="PSUM") as pp,
        ):
            # tiny dummy activation with no DMA deps: the table-load
            # pass places the ~1.3us ACT_TABLE_LOAD before it, pulling the
            # load into the idle preamble window off the critical path
            pre = wp.tile([1, 8], dt.bfloat16, name="act_pre")
            nc.vector.memset(pre, 0.0)
            nc.scalar.activation(pre, pre, AFT.Sigmoid)

            # ---- resident loads: noise weights + tile-0 x first so the
            # PE can start on the noise branch while w0g still loads ----
            xTs = [wp.tile([F, RT], dt.bfloat16, name=f"xT_sb{t}")
                   for t in range(NT)]
            nc.sync.dma_start(out=xTs[0], in_=d_xT[:, 0:RT])
            w0n = wp.tile([F, 3 * NH], dt.bfloat16, name="w0n_sb")
            nc.sync.dma_start(out=w0n, in_=d_w0n[:, :])
            bn0 = wp.tile([128, 3 * CN], dt.float32, name="bn0_sb")
            nc.sync.dma_start(out=bn0, in_=d_bn0[:, :])
            w0g = wp.tile([F, 3 * GH], dt.bfloat16, name="w0g_sb")
            nc.sync.dma_start(out=w0g, in_=d_w0g[:, :])
            bg0 = wp.tile([128, 3 * CG], dt.float32, name="bg0_sb")
            nc.sync.dma_start(out=bg0, in_=d_bg0[:, :])
            w1n = [wp.tile([128, 3 * NH], dt.bfloat16, name=f"w1n_sb{k}")
                   for k in range(CN)]
            for k in range(CN):
                nc.sync.dma_start(out=w1n[k], in_=d_w1n[k * 128:(k + 1) * 128, :])
            bn1 = wp.tile([128, 3 * CN], dt.float32, name="bn1_sb")
            nc.sync.dma_start(out=bn1, in_=d_bn1[:, :])
            w1g = [wp.tile([128, 3 * GH], dt.bfloat16, name=f"w1g_sb{k}")
                   for k in range(CG)]
            for k in range(CG):
                nc.sync.dma_start(out=w1g[k], in_=d_w1g[k * 128:(k + 1) * 128, :])
            bg1 = wp.tile([128, 3 * CG], dt.float32, name="bg1_sb")
            nc.sync.dma_start(out=bg1, in_=d_bg1[:, :])
            for t in range(1, NT):
                nc.sync.dma_start(out=xTs[t], in_=d_xT[:, t * RT:(t + 1) * RT])
            wmu = wp.tile([128, CG], dt.bfloat16, name="wmu_sb")
            nc.sync.dma_start(out=wmu, in_=d_wmu[:, :])
            wsig = wp.tile([128, CN], dt.bfloat16, name="wsig_sb")
            nc.sync.dma_start(out=wsig, in_=d_wsig[:, :])

            def layer_group(t, C, rhs_list, w_list, b_sb, out_tag, form, relu):
                """One full LSTM step (all C hidden chunks) for RT rows.
                Returns per-chunk emission thunks; no cross-chunk barriers,
                so the PE stream never stalls behind a batched activation.
                rhs_list may contain deferred lists filled by earlier thunks."""
                nk = len(rhs_list)
                hs_out = [None] * C

                def chunk(c):
                    ps = []
                    for gi in range(3):  # i, g, o
                        p = pp.tile([128, RT], dt.float32, tag="ps", bufs=4,
                                    name=f"p_{out_tag}_{t}_{c}_{gi}")
                        mcol = (gi * C + c) * 128
                        for k in range(nk):
                            for h in range(RT // HALF):
                                hs = slice(h * HALF, (h + 1) * HALF)
                                nc.tensor.matmul(
                                    p[:, hs],
                                    w_list[k][:, mcol:mcol + 128],
                                    rhs_list[k][:, hs],
                                    start=(k == 0), stop=(k == nk - 1),
                                )
                        ps.append(p)
                    pi, pg, po = ps
                    ti = gp.tile([128, RT], dt.bfloat16, tag="ti", bufs=5,
                                 name=f"ti_{out_tag}_{t}_{c}")
                    nc.scalar.activation(ti, pi, AFT.Sigmoid, bias=b_sb[:, c:c + 1])
                    tg = gp.tile([128, RT], dt.bfloat16, tag="tg", bufs=5,
                                 name=f"tg_{out_tag}_{t}_{c}")
                    nc.scalar.activation(tg, pg, AFT.Tanh,
                                         bias=b_sb[:, C + c:C + c + 1])
                    to = gp.tile([128, RT], dt.bfloat16, tag="to", bufs=7,
                                 name=f"to_{out_tag}_{t}_{c}")
                    nc.scalar.activation(to, po, AFT.Sigmoid,
                                         bias=b_sb[:, 2 * C + c:2 * C + c + 1])
                    cc = gp.tile([128, RT], dt.bfloat16, tag="cc", bufs=4,
                                 name=f"cc_{out_tag}_{t}_{c}")
                    nc.vector.tensor_mul(cc, ti, tg)
                    th = gp.tile([128, RT], dt.bfloat16, tag="th", bufs=4,
                                 name=f"th_{out_tag}_{t}_{c}")
                    if form == "sq":
                        if relu:
                            # relu(sig(o)*tanh(c)) == sig(o)*tanh(relu(c))
                            nc.vector.tensor_scalar_max(cc, cc, 0.0)
                        tq = gp.tile([128, RT], dt.bfloat16, tag="pta", bufs=3,
                                     name=f"tq_{out_tag}_{t}_{c}")
                        nc.vector.tensor_mul(tq, cc, cc)
                        qq = gp.tile([128, RT], dt.bfloat16, tag="ptb", bufs=3,
                                     name=f"qq_{out_tag}_{t}_{c}")
                        nc.vector.tensor_scalar(qq, tq, PD1, PD0, op0=ALU.mult,
                                                op1=ALU.add)
                        nc.vector.tensor_mul(th, qq, cc)
                    else:
                        # layer-1 poly: th = (A0 + A1*c)*c, one fused
                        # tensor_scalar + one tensor_tensor. No max needed:
                        # for c in [-1,0), A0+A1*c > 0 so sign(h) == sign(c)
                        # and the final relu on h zeroes it, matching
                        # tanh(relu(c)) == 0; for c >= 0 it is the fitted
                        # c*(A0+A1*c) quad exactly.
                        a0, a1 = form
                        w = gp.tile([128, RT], dt.bfloat16, tag="ptb", bufs=3,
                                    name=f"w_{out_tag}_{t}_{c}")
                        nc.vector.tensor_scalar(w, cc, a1, a0,
                                                op0=ALU.mult, op1=ALU.add)
                        nc.vector.tensor_mul(th, w, cc)
                    h = hp.tile([128, RT], dt.bfloat16, tag=out_tag,
                                bufs=(3 if relu else 2) * C,
                                name=f"h_{out_tag}_{t}_{c}")
                    nc.vector.tensor_mul(h, to, th)
                    if form != "sq" and relu:
                        nc.vector.tensor_scalar_max(h, h, 0.0)
                    hs_out[c] = h

                thunks = [partial(chunk, c) for c in range(C)]
                return thunks, hs_out

            def tail_thunks(t, r1g, r1n, last=False):
                # single-row sums: mu[row] = wmu . r1g[:, row] at partition 0,
                # sig[row] = wsig . r1n[:, row] at partition 32 of the same
                # PSUM tile -> one ACT staging copy serves both outputs.
                box = {}

                def emit_mu():
                    pz = pp.tile([33, RT], dt.float32, tag="ps", bufs=4,
                                 name=f"pz_{t}")
                    box["pz"] = pz
                    for k in range(CG):
                        for h in range(RT // HALF):
                            hs = slice(h * HALF, (h + 1) * HALF)
                            nc.tensor.matmul(pz[0:1, hs], wmu[:, k:k + 1],
                                             r1g[k][:, hs],
                                             start=(k == 0), stop=(k == CG - 1))
                    if last:
                        stm = gp.tile([1, RT], dt.float32, tag="stm", bufs=1,
                                      name=f"stm_{t}")
                        nc.scalar.copy(stm, pz[0:1, :])
                        nc.sync.dma_start(out=d_mus[:, t * RT:(t + 1) * RT],
                                          in_=stm)

                def emit_sig():
                    pz = box["pz"]
                    for k in range(CN):
                        for h in range(RT // HALF):
                            hs = slice(h * HALF, (h + 1) * HALF)
                            nc.tensor.matmul(pz[32:33, hs], wsig[:, k:k + 1],
                                             r1n[k][:, hs],
                                             start=(k == 0), stop=(k == CN - 1),
                                             tile_position=(0, 32))
                    if last:
                        sts = gp.tile([1, RT], dt.float32, tag="sts", bufs=1,
                                      name=f"sts_{t}")
                        nc.scalar.copy(sts, pz[32:33, :])
                        nc.sync.dma_start(out=d_zs[:, t * RT:(t + 1) * RT],
                                          in_=sts)
                    else:
                        st = gp.tile([33, RT], dt.float32, tag="st", bufs=2,
                                     name=f"st_{t}")
                        # on the DVE: ACT and PE co-bind, the DVE has slack
                        nc.vector.tensor_copy(st, pz)
                        nc.sync.dma_start(out=d_mus[:, t * RT:(t + 1) * RT],
                                          in_=st[0:1, :])
                        nc.sync.dma_start(out=d_zs[:, t * RT:(t + 1) * RT],
                                          in_=st[32:33, :])

                return emit_mu, emit_sig

            # Software pipeline with one-tile skew: tile t's layer-0 work
            # (ACT-heavy, PE-light) is emitted interleaved with tile t-1's
            # layer-1 work (PE-heavy, ACT-light), so both engine queues stay
            # dense and the PE never idles long enough to lose HAM warmth.
            light, heavy, tails = [], [], []
            for t in range(NT):
                b_th, h0n = layer_group(t, CN, [xTs[t]], [w0n], bn0, "h0n",
                                        "sq", False)
                a_th, h0g = layer_group(t, CG, [xTs[t]], [w0g], bg0, "h0g",
                                        "sq", False)
                c_th, r1g = layer_group(t, CG, h0g, w1g, bg1, "r1g",
                                        ABg, True)
                d_th, r1n = layer_group(t, CN, h0n, w1n, bn1, "r1n",
                                        ABn, True)
                mu_th, sig_th = tail_thunks(t, r1g, r1n, last=t == NT - 1)
                # tile 0 runs standalone: noise first so the PE starts while
                # w0g still loads; later tiles interleave against the
                # PE-heavy layer-1 stream, where global L0 (ACT-heavy) must
                # lead to keep both queues dense
                light.append(b_th + a_th if t == 0 else a_th + b_th)
                heavy.append(c_th + d_th)
                tails.append([mu_th, sig_th])

            def interleave(xs, ys):
                out = []
                n = max(len(xs), len(ys))
                for i in range(n):
                    if i < len(xs):
                        out.append(xs[i])
                    if i < len(ys):
                        out.append(ys[i])
                return out

            # mu/sig tails have no consumers, so they are emitted a full
            # round after their r1 inputs: their matmuls are always
            # instantly ready and never head-of-line-block the PE FIFO
            for th in light[0]:
                th()
            for r in range(1, NT):
                stream = (tails[r - 2] if r >= 2 else []) + heavy[r - 1]
                for th in interleave(stream, light[r]):
                    th()
            # final drain: r1g chunks first, then r1n with the tails slotted
            # as soon as their inputs are ready
            fin = heavy[NT - 1]
            for th in fin[:CG]:
                th()
            tails[NT - 2][0]()        # old mu (inputs long ready)
            fin[CG]()                 # r1n chunk 0
            tails[NT - 2][1]()        # old sig + copy + dma
            fin[CG + 1]()             # r1n chunk 1
            tails[NT - 1][0]()        # this tile's mu (r1g ready)
            tails[NT - 1][1]()        # final sig + copy + dma

    nc.compile()
    return nc


def _pack_lstm_weights(W, b, H):
    """Drop the f gate; pack [i, g, o] along the output dim.
    Returns lhsT (K, 3H) bf16 and bias tile (128, 3H/128) f32."""
    idx = np.r_[0:H, 2 * H:3 * H, 3 * H:4 * H]
    Wp = W[idx]                      # (3H, K)
    bp = b[idx]                      # (3H,)
    lhsT = np.ascontiguousarray(Wp.T).astype(BF16)
    btile = np.ascontiguousarray(bp.reshape(3 * H // 128, 128).T).astype(np.float32)
    return lhsT, btile


def _make_in_maps(inputs):
    """Host-side packing: shard X, drop f-gates, fold aff into one dot.
    Returns (per-core input maps, summed aff bias, noise bias)."""
    X = np.asarray(inputs["X"], np.float32)
    g_Wih0 = np.asarray(inputs["g_Wih0"], np.float32)
    g_b0 = np.asarray(inputs["g_b0"], np.float32)
    g_Wih1 = np.asarray(inputs["g_Wih1"], np.float32)
    g_b1 = np.asarray(inputs["g_b1"], np.float32)
    aff_W = np.asarray(inputs["aff_W"], np.float32)
    aff_b = np.asarray(inputs["aff_b"], np.float32)
    n_Wih0 = np.asarray(inputs["n_Wih0"], np.float32)
    n_b0 = np.asarray(inputs["n_b0"], np.float32)
    n_Wih1 = np.asarray(inputs["n_Wih1"], np.float32)
    n_b1 = np.asarray(inputs["n_b1"], np.float32)
    noise_W = np.asarray(inputs["noise_W"], np.float32)
    noise_b = np.asarray(inputs["noise_b"], np.float32)

    w0g, bg0 = _pack_lstm_weights(g_Wih0, g_b0, GH)
    w1g, bg1 = _pack_lstm_weights(g_Wih1, g_b1, GH)
    w0n, bn0 = _pack_lstm_weights(n_Wih0, n_b0, NH)
    w1n, bn1 = _pack_lstm_weights(n_Wih1, n_b1, NH)

    wm = aff_W.sum(axis=0)                     # (GH,)
    wmu = np.ascontiguousarray(wm.reshape(GH // 128, 128).T).astype(BF16)
    b_mu = float(aff_b.sum())
    ws = noise_W[0]                            # (NH,)
    wsig = np.ascontiguousarray(ws.reshape(NH // 128, 128).T).astype(BF16)
    b_sig = float(noise_b[0])

    Xf = X.reshape(ROWS, F)
    shared = {
        "w0g": w0g, "w1g": w1g, "w0n": w0n, "w1n": w1n,
        "wmu": wmu, "wsig": wsig,
        "bg0": bg0, "bg1": bg1, "bn0": bn0, "bn1": bn1,
    }
    in_maps = []
    for c in range(NCORES):
        xc = np.ascontiguousarray(
            Xf[c * RPC:(c + 1) * RPC].T).astype(BF16)    # (F, RPC)
        in_maps.append({"xT": xc, **shared})
    return in_maps, b_mu, b_sig


def kernel(**inputs):
    from concourse.bass_utils import run_bass_kernel_spmd

    in_maps, b_mu, b_sig = _make_in_maps(inputs)
    if "nc" not in _CACHE:
        _CACHE["nc"] = _build_program()
    nc = _CACHE["nc"]

    res = run_bass_kernel_spmd(nc, in_maps, list(range(NCORES)))

    mus = np.empty(ROWS, np.float32)
    zs = np.empty(ROWS, np.float32)
    for c in range(NCORES):
        mus[c * RPC:(c + 1) * RPC] = res.results[c]["mus_o"][0]
        zs[c * RPC:(c + 1) * RPC] = res.results[c]["zs_o"][0]
    # device outputs the raw row sums; the constant aff bias, the softplus
    # epilogue over 32k scalars, and the +1e-6 epsilon fold on host
    mus = (mus + b_mu).reshape(NTS, NPER)
    sig = (np.logaddexp(0.0, zs + b_sig).astype(np.float32) + 1e-6).reshape(NTS, NPER)
    return mus, sig
